# revision 23
# baseline (speedup 1.0000x reference)
"""BertSelfAttention on 8 Trainium2 NeuronCores (Bass/Tile).

Sharding: tensor-parallel over heads. 16 heads / 8 cores = 2 heads (128
head-dim columns) per core. Each core computes the Q/K/V projections for
its 128 output dims over all 4096 tokens, then attention for its 2 heads
over both batches, producing a [4096, 128] token-major slice of the
output. The host transposes hidden_states once (and casts to bf16), feeds
every core the same [1024, 4096] activation matrix plus its private
weight slice, and reassembles the full [2, 2048, 1024] output by
concatenating the 8 column slices.

Device-side layout (driven by the cost model: a matmul charges
out_free_size cycles per contraction pass, so output partitions must be
maximized on every matmul):
  - Q,K are produced d-major [dim, token] in fp32r. QK^T yields scores
    TRANSPOSED, S^T[key, query], keys on partitions: 512-row matmuls at
    full fp32r rate.
  - exp() runs on ScalarE (the critical engine: 16.8M exps = ~133us);
    output is fp16 (exp(s/8) spans ~[0.1, 7.4], comfortably fp16).
  - V is projected d-major (fp16) then PE-transposed to token-major
    tiles with a ones-column appended per head: vtm[(b,kt)] =
    [128 keys, 65+65].
  - PV is oriented with QUERIES on output partitions: stationary =
    probs slice [128 keys, 128 queries], moving = vtm [128 keys, 65]
    -> ctx^T psum [128 q, 65] where column 64 is the softmax
    denominator. 65 charged rows/matmul instead of 512: PV costs 67k
    rows instead of 131k, and the denominator is per-PARTITION, so
    normalization is a plain DVE tensor_scalar multiply -- no
    reciprocal-broadcast matmuls at all.
  - hidden/weights are bf16 (halves the dominant 16MB input DMA);
    Q/K/scores stay fp32 so softmax inputs are near-exact.
  - Emission order IS the per-engine schedule. The kernel is
    software-pipelined so ScalarE (exp) never starves: each chunk's
    QK for chunk N+1 is emitted before chunk N's PV, projections ride
    as fillers in PE slack, and each block's first QK is hoisted into
    the previous block's last chunk.
"""

import numpy as np

import concourse.tile as tile
from concourse import bacc, mybir
from concourse.bass_utils import run_bass_kernel_spmd
from concourse.masks import make_identity

# Problem shape (hardcoded; harness contract)
B, S, H = 2, 2048, 1024
NUM_HEADS, DH = 16, 64
NCORES = 8
T = B * S                 # 4096 tokens total
D = H // NCORES           # 128 output dims per core (2 heads)
KC = H // 128             # 8 contraction chunks for projections
QB = 512                  # query-block width
NQB = S // QB             # 4 query blocks per batch
NTB = T // QB             # 8 projection token-blocks
NKT = S // 128            # 16 key tiles per batch
KTC = 2                   # key tiles per exp chunk (psum-budget limited)
NQT = QB // 128           # 4 query sub-tiles of 128 per block
SCALE = 1.0 / float(np.sqrt(DH))

F32 = mybir.dt.float32
F32R = mybir.dt.float32r
F16 = mybir.dt.float16
BF16 = mybir.dt.bfloat16
EXP = mybir.ActivationFunctionType.Exp


def build(use_mask: bool, use_bias: bool, reps: int = 1):
    nc = bacc.Bacc("TRN2", target_bir_lowering=False)

    hT = nc.dram_tensor("hT", [H, T], BF16, kind="ExternalInput")
    # weights pre-shuffled on host to [128, KC*D] so the DMA moves 2KB
    # contiguous lines per partition (full bus efficiency)
    wq = nc.dram_tensor("wq", [128, KC * D], BF16, kind="ExternalInput")
    wk = nc.dram_tensor("wk", [128, KC * D], BF16, kind="ExternalInput")
    wv = nc.dram_tensor("wv", [128, KC * D], BF16, kind="ExternalInput")
    if use_bias:
        bq = nc.dram_tensor("bq", [D, 1], F32, kind="ExternalInput")
        bk = nc.dram_tensor("bk", [D, 1], F32, kind="ExternalInput")
        bv = nc.dram_tensor("bv", [D, 1], F32, kind="ExternalInput")
    if use_mask:
        # host pre-transposes to [128, B, NKT] so the DMA is contiguous
        mask = nc.dram_tensor("mask", [128, B, NKT], F32, kind="ExternalInput")
    # token-major output: rows are tokens, cols are this core's 128 dims
    out = nc.dram_tensor("out", [T, D], F32, kind="ExternalOutput")

    with tile.TileContext(nc) as tc:
        with (
            tc.tile_pool(name="consts", bufs=1) as consts,
            tc.tile_pool(name="qkv", bufs=1) as qkvp,
            tc.tile_pool(name="ht", bufs=2) as htp,
            tc.tile_pool(name="vtm", bufs=2) as vtmp,
            tc.tile_pool(name="e", bufs=22) as ep,
            tc.tile_pool(name="small", bufs=4) as smallp,
            tc.tile_pool(name="pp", bufs=2, space="PSUM") as pp,
            tc.tile_pool(name="qk", bufs=2, space="PSUM") as qkp,
            tc.tile_pool(name="pv", bufs=2, space="PSUM") as pvp,
        ):
            # ---- constants ----
            w_sb = {}
            b_sb = {}
            w_dram = {"q": wq, "k": wk, "v": wv}
            for name in ("q", "k", "v"):
                w_sb[name] = consts.tile(
                    [128, KC, D], BF16, tag=f"w{name}", name=f"w{name}"
                )

            def load_w(name):
                nc.sync.dma_start(
                    out=w_sb[name][:],
                    in_=w_dram[name].rearrange("p (kc d) -> p kc d", kc=KC),
                )

            # DMA order: the sync queue drains in emission order. The first
            # K-projection half needs wk + the first half (kc 0..3) of hT
            # block 0, the Q half needs wq; wv is only needed later.
            hts = {}
            hT_r = hT.rearrange("(kc p) t -> p kc t", p=128)

            # issue the first three DMAs from three different engine queues
            # so their descriptor-generation pipelines overlap and the first
            # transfer starts ~0.6us earlier
            load_w("k")
            ht0 = htp.tile([128, KC, QB], BF16, tag="ht", name="ht")
            nc.sync.dma_start(
                out=ht0[:, 0 : KC // 2, :], in_=hT_r[:, 0 : KC // 2, 0:QB]
            )
            load_w("q")
            nc.sync.dma_start(
                out=ht0[:, KC // 2 :, :], in_=hT_r[:, KC // 2 :, 0:QB]
            )
            hts[0] = ht0
            load_w("v")

            if use_bias:
                for name, bt in (("q", bq), ("k", bk), ("v", bv)):
                    b_t = consts.tile([128, 1], F32, tag=f"b{name}")
                    nc.sync.dma_start(out=b_t[:], in_=bt[:])
                    b_sb[name] = b_t
            ident = consts.tile([128, 128], F16, tag="ident")
            make_identity(nc, ident[:])
            ones_st = consts.tile([128, 2], F16, tag="onesst")
            nc.vector.memset(ones_st[:], 1.0)
            # constant zero tile for p-state keeper matmuls (no DMA dep)
            dum = consts.tile([128, 128], BF16, tag="dum")
            nc.vector.memset(dum[:], 0.0)
            if use_mask:
                mask_sb = consts.tile([128, B, NKT], F32, tag="mask")
                nc.sync.dma_start(out=mask_sb[:], in_=mask[:])

            # per-token-block tiles so attention dependencies are precise
            Qts = [qkvp.tile([128, QB], F32R, tag=f"Qd{i}", name=f"Qd{i}") for i in range(NTB)]
            Kts = [qkvp.tile([128, QB], F32R, tag=f"Kd{i}", name=f"Kd{i}") for i in range(NTB)]
            Vts = [qkvp.tile([128, QB], F16, tag=f"Vd{i}", name=f"Vd{i}") for i in range(NTB)]

            def proj_load(tb):
                t0 = tb * QB
                ht_t = htp.tile([128, KC, QB], BF16, tag="ht", name="ht")
                hT_r = hT.rearrange("(kc p) t -> p kc t", p=128)
                nc.sync.dma_start(out=ht_t[:], in_=hT_r[:, :, t0 : t0 + QB])
                return ht_t

            def proj_group(tb, name):
                dest = {"q": Qts, "k": Kts, "v": Vts}[name][tb]
                ps = pp.tile([128, QB], F32, tag="pp", name="ps")
                for kc in range(KC):
                    nc.tensor.matmul(
                        ps[:],
                        w_sb[name][:, kc, :],
                        hts[tb][:, kc, :],
                        start=(kc == 0),
                        stop=(kc == KC - 1),
                    )
                if use_bias:
                    nc.vector.tensor_scalar_add(dest[:], ps[:], b_sb[name][:])
                else:
                    nc.vector.tensor_copy(dest[:], ps[:])

            # V token-major tiles per k-tile, 130 wide:
            # [h0 dims 0-63 | ones | h1 dims 0-63 | ones]; the ones column
            # makes the PV matmul also produce the softmax denominator in
            # psum column 64 (per query partition).
            vtms = {}

            def transpose_v(b, kt):
                g0 = b * S + kt * 128
                tbi, off = divmod(g0, QB)
                vt = vtmp.tile([128, 130], F16, tag=f"vtm{kt}", name=f"vtm{kt}")
                nc.vector.tensor_copy(
                    vt[:, 64::65].rearrange("p (a o) -> p a o", o=1),
                    ones_st[:, 0:2].rearrange("p (a o) -> p a o", o=1),
                )
                tps = pp.tile([128, 128], F16, tag="pp", name="tps")
                nc.tensor.transpose(tps[:], Vts[tbi][:, off : off + 128], ident[:])
                nc.vector.tensor_copy(
                    vt[:].rearrange("p (g c) -> p g c", g=2)[:, :, 0:64],
                    tps.rearrange("p (g c) -> p g c", g=2),
                )
                vtms[(b, kt)] = vt

            # ---- explicit software-pipelined emission ----

            def attn_open(b, qb):
                # per head: one bank [128, NQT, 65] = ctx^T rows (queries on
                # partitions) + denominator column 64, 4 query-subtile
                # accumulation groups. start_tensor_calc resets the WHOLE
                # psum bank on hardware, so the sub-bank qt groups cannot
                # each use start=True: zero the bank once (DVE) and let
                # every PV matmul accumulate.
                ctx_ps = []
                for h in range(2):
                    t = pvp.tile([128, NQT, 65], F32, tag="ctx", name=f"ctx{h}")
                    nc.vector.memset(t[:], 0.0)
                    ctx_ps.append(t)
                return (b, qb, ctx_ps)

            def attn_qk(state, ktc):
                """Emit one chunk's QK^T matmuls. Returns score psum tiles."""
                b, qb, _ = state
                q0 = b * S + qb * QB
                sps = [
                    qkp.tile([128, KTC, QB], F32, tag="sps", name=f"sps{h}")
                    for h in range(2)
                ]
                for j in range(KTC):
                    kt = ktc * KTC + j
                    tbi, off = divmod(b * S + kt * 128, QB)
                    for h in (0, 1):
                        nc.tensor.matmul(
                            sps[h][:, j, :],
                            Kts[tbi][h * 64 : (h + 1) * 64, off : off + 128],
                            Qts[q0 // QB][h * 64 : (h + 1) * 64, :],
                            start=True,
                            stop=True,
                        )
                return sps

            def attn_exp(state, ktc, sps):
                """exp on ScalarE -> fp16 prob tiles (the critical engine)."""
                b, qb, _ = state
                ets = []
                for h in (0, 1):
                    et = ep.tile([128, KTC, QB], F16, tag="e", name=f"et{h}")
                    if use_mask:
                        for j in range(KTC):
                            kt = ktc * KTC + j
                            nc.scalar.activation(
                                et[:, j, :],
                                sps[h][:, j, :],
                                EXP,
                                bias=mask_sb[:, b, kt : kt + 1],
                                scale=SCALE,
                            )
                    else:
                        nc.scalar.activation(et[:], sps[h][:], EXP, scale=SCALE)
                    ets.append(et)
                return ets

            def attn_pv(state, ktc, ets):
                b, qb, ctx_ps = state
                for j in range(KTC):
                    kt = ktc * KTC + j
                    for h in (0, 1):
                        for qt in range(NQT):
                            nc.tensor.matmul(
                                ctx_ps[h][:, qt, :],
                                ets[h][:, j, qt * 128 : (qt + 1) * 128],
                                vtms[(b, kt)][:, h * 65 : (h + 1) * 65],
                                start=False,
                                stop=(kt == NKT - 1),
                                skip_group_check=True,
                            )

            # exp-stream / pv-stream decoupling: exps for block E run while
            # PVs for the PREVIOUS block P consume parked prob tiles. The
            # two streams only meet through the et pool, so ScalarE never
            # waits for PV progress and the PE always has parked PV work.
            park = {}  # (b, qb) -> {ktc: ets}

            def exp_chunk(b, qb, ktc):
                stv = (b, qb, None)
                sps = attn_qk(stv, ktc)
                ets = attn_exp(stv, ktc, sps)
                park.setdefault((b, qb), {})[ktc] = ets

            def pv_chunk(state, ktc):
                b, qb, _ = state
                attn_pv(state, ktc, park[(b, qb)].pop(ktc))

            def attn_close(state, last=False):
                """Emit the reciprocals now (DVE, off the PE/ACT paths) and
                return a thunk with the normalize + output-DMA tail. For the
                final block (no more exps) half the normalize runs on the
                now-idle ScalarE to shorten the serial tail."""
                b, qb, ctx_ps = state
                recs = []
                for h in (0, 1):
                    rec = smallp.tile([128, NQT, 1], F32, tag="rec", name=f"rec{h}")
                    nc.vector.reciprocal(rec[:], ctx_ps[h][:, :, 64:65])
                    recs.append(rec)

                def finish():
                    ot = smallp.tile([128, NQT, 2, 64], F32, tag="ot", name="ot", bufs=2)
                    for qt in range(NQT):
                        for h in (0, 1):
                            if last and h == 1:
                                nc.scalar.mul(
                                    ot[:, qt, h, :],
                                    ctx_ps[h][:, qt, 0:64],
                                    recs[h][:, qt, :],
                                )
                            else:
                                nc.vector.tensor_scalar_mul(
                                    ot[:, qt, h, :],
                                    ctx_ps[h][:, qt, 0:64],
                                    recs[h][:, qt, :],
                                )
                    tb0 = (b * S + qb * QB) // 128
                    out_r = out.rearrange("(tb p) d -> p tb d", p=128)
                    nc.sync.dma_start(
                        out=out_r[:, tb0 : tb0 + NQT, :],
                        in_=ot[:].rearrange("p a b c -> p a (b c)"),
                    )

                return finish

            # --- the pipeline driver ---
            # PE p-state: the cost model drops the PE clock whenever the
            # engine idles (the ramp needs ~3us of continuous execution to
            # reach full rate). Tiny dummy matmuls on a memset tile (no DMA
            # dependency) keep the PE continuously busy from t~0.3us across
            # the initial DMA waits so the first REAL matmuls run at full
            # rate immediately.
            warm = qkp.tile([128, KTC, QB], F32, tag="sps", name="warm")

            def dummies(n):
                def f():
                    for _ in range(n):
                        nc.tensor.matmul(
                            warm[0:64, 0, 0:64],
                            dum[:, 0:64],
                            dum[:, 0:64],
                            start=True,
                            stop=True,
                        )

                return f

            dummies(110)()

            def load(tb):
                def f():
                    hts[tb] = proj_load(tb)

                return f

            def grp(tb, n):
                return lambda: proj_group(tb, n)

            def grp_halves(tb, n):
                """Split one projection group into two 4-kc emission halves
                (same psum accumulation bracket) so a filler never injects
                more than ~0.9us of PE work between attention chunks."""
                stash = {}

                def h1():
                    dest = {"q": Qts, "k": Kts, "v": Vts}[n][tb]
                    ps = pp.tile([128, QB], F32, tag="pp", name="ps")
                    stash["ps"], stash["dest"] = ps, dest
                    for kc in range(KC // 2):
                        nc.tensor.matmul(
                            ps[:],
                            w_sb[n][:, kc, :],
                            hts[tb][:, kc, :],
                            start=(kc == 0),
                            stop=False,
                        )

                def h2():
                    ps, dest = stash["ps"], stash["dest"]
                    for kc in range(KC // 2, KC):
                        nc.tensor.matmul(
                            ps[:],
                            w_sb[n][:, kc, :],
                            hts[tb][:, kc, :],
                            start=False,
                            stop=(kc == KC - 1),
                        )
                    if use_bias:
                        nc.vector.tensor_scalar_add(dest[:], ps[:], b_sb[n][:])
                    else:
                        nc.vector.tensor_copy(dest[:], ps[:])

                return h1, h2

            def seq(*fs):
                def f():
                    for g in fs:
                        g()

                return f

            def tr2(b_, k_):
                def f():
                    transpose_v(b_, k_)
                    transpose_v(b_, k_ + 1)

                return f

            def dmy():
                # p-state keeper for chunks with no real filler work: ~640ns
                # of junk matmuls into a rotating pp slot so the PE never
                # idles (an idle PE drops to the slow clock for ~3us).
                dps = pp.tile([64, QB], F32, tag="pp", name="dps")
                for _ in range(3):
                    nc.tensor.matmul(
                        dps[:],
                        dum[:, 0:64],
                        w_sb["q"][:, 0:4, :].rearrange("p a b -> p (a b)"),
                        start=True,
                        stop=True,
                    )

            none = lambda: None

            def emit_pass():
                # --- priming: exps for (0,0) AND (0,1) interleave as K/V
                # blocks get projected, so ScalarE is nearly saturated even
                # while the PE is projection-bound. (0,1)'s probs are parked
                # in the et pool; its PVs run in the first pair below.
                # Dummy batches bridge DMA waits so the PE p-state never
                # drops before/between the first real matmuls.
                st00 = attn_open(0, 0)
                k1, k2 = grp_halves(0, "k")
                q1, q2 = grp_halves(0, "q")
                v1, v2 = grp_halves(0, "v")
                k1(); q1()
                dummies(10)()
                k2(); q2()
                dummies(6)()
                exp_chunk(0, 0, 0)
                seq(v1, v2, tr2(0, 0))()
                exp_chunk(0, 0, 1)
                tr2(0, 2)()
                pv_chunk(st00, 0)
                for tb in range(1, NTB // B):
                    load(tb)()
                    grp(tb, "k")()
                    grp(tb, "v")()
                    for kt in range(4 * tb, 4 * tb + 4):
                        transpose_v(0, kt)
                    if tb in (1, 2):
                        grp(tb, "q")()
                    if tb == NTB // B - 1:
                        load(NTB // B)()
                    exp_chunk(0, 0, 2 * tb)
                    exp_chunk(0, 1, 2 * (tb - 1))
                    pv_chunk(st00, 2 * tb - 1)
                    exp_chunk(0, 0, 2 * tb + 1)
                    exp_chunk(0, 1, 2 * (tb - 1) + 1)
                    pv_chunk(st00, 2 * tb)
                q31, q32 = grp_halves(NTB // B - 1, "q")
                exp_chunk(0, 1, 6)
                q31()
                exp_chunk(0, 1, 7)
                q32()
                pv_chunk(st00, 7)
                fin = attn_close(st00)

                # --- steady state: pairs (P, E) — P's parked PVs + E's
                # QK/exps per slot; batch-1 projections/transposes/loads
                # ride as per-slot fillers, q-projections just-in-time,
                # dummy batches keep the PE p-state up where there's no
                # real filler work.
                q41, q42 = grp_halves(4, "q"); k41, k42 = grp_halves(4, "k"); v41, v42 = grp_halves(4, "v")
                q51, q52 = grp_halves(5, "q"); k51, k52 = grp_halves(5, "k"); v51, v52 = grp_halves(5, "v")
                q61, q62 = grp_halves(6, "q"); k61, k62 = grp_halves(6, "k"); v61, v62 = grp_halves(6, "v")
                q71, q72 = grp_halves(7, "q"); k71, k72 = grp_halves(7, "k"); v71, v72 = grp_halves(7, "v")
                pairs = [
                    ((0, 1), (0, 2), [q41, q42, k41, k42, v41,
                                      seq(v42, load(5)), tr2(1, 0), tr2(1, 2)]),
                    ((0, 2), (0, 3), [q51, q52, k51, k52, v51,
                                      seq(v52, load(6)), tr2(1, 4),
                                      seq(tr2(1, 6), load(7))]),
                    ((0, 3), (1, 0), [k61, k62, v61, v62, k71, k72,
                                      tr2(1, 8), tr2(1, 10)]),
                    ((1, 0), (1, 1), [q61, q62, v71, seq(v72, tr2(1, 12)),
                                      tr2(1, 14), q71, q72, dmy]),
                    ((1, 1), (1, 2), [dmy] * 8),
                ]
                for (pb, pqb), (eb, eqb), fillers in pairs:
                    stP = attn_open(pb, pqb)
                    for c in range(NKT // KTC):
                        fillers[c]()
                        exp_chunk(eb, eqb, c)
                        if c == 0:
                            fin()
                        pv_chunk(stP, c)
                    fin = attn_close(stP)

                # --- final pair: P=(1,2) drains its parked PVs early (2 per
                # slot), closes mid-pair, then E=(1,3) opens (ctx banks
                # freed) and drains its own parked PVs in the last slots so
                # the tail after the final exp is just 2 PV batches + the
                # (ACT/DVE-split) normalize.
                stP = attn_open(1, 2)
                stE = None
                finP = None
                for c in range(NKT // KTC):
                    exp_chunk(1, 3, c)
                    if c == 0:
                        fin()
                    if c < 4:
                        pv_chunk(stP, 2 * c)
                        pv_chunk(stP, 2 * c + 1)
                    elif c == 4:
                        finP = attn_close(stP)
                        dmy()
                    elif c == 5:
                        finP()
                        dmy()
                    elif c == 6:
                        stE = attn_open(1, 3)
                        pv_chunk(stE, 0)
                        pv_chunk(stE, 1)
                        pv_chunk(stE, 2)
                    else:
                        pv_chunk(stE, 3)
                        pv_chunk(stE, 4)
                        pv_chunk(stE, 5)
                pv_chunk(stE, 6)
                pv_chunk(stE, 7)
                finE = attn_close(stE, last=True)
                finE()

            for _ in range(reps):
                emit_pass()
    nc.compile()
    return nc


_BUILD_CACHE = {}


def _get_nc(use_mask, use_bias):
    key = (use_mask, use_bias)
    if key not in _BUILD_CACHE:
        _BUILD_CACHE[key] = build(use_mask, use_bias)
    return _BUILD_CACHE[key]


def kernel(hidden_states, attention_mask, Wq, bq, Wk, bk, Wv, bv, _trace=False):
    import ml_dtypes

    bf16 = ml_dtypes.bfloat16

    hidden = np.ascontiguousarray(np.asarray(hidden_states, dtype=np.float32))
    mask = np.asarray(attention_mask, dtype=np.float32).reshape(B, S)
    Wq = np.asarray(Wq, dtype=np.float32)
    Wk = np.asarray(Wk, dtype=np.float32)
    Wv = np.asarray(Wv, dtype=np.float32)
    bq = np.asarray(bq, dtype=np.float32)
    bk = np.asarray(bk, dtype=np.float32)
    bv = np.asarray(bv, dtype=np.float32)

    use_mask = bool(np.any(mask != 0.0))
    use_bias = bool(np.any(bq != 0.0) or np.any(bk != 0.0) or np.any(bv != 0.0))
    nc = _get_nc(use_mask, use_bias)

    hT = np.ascontiguousarray(hidden.reshape(T, H).T).astype(bf16)  # [H, T]

    def pack_w(w, sl):
        # [H, D] slice -> [128, KC*D]: row p holds w[kc*128+p, :] for all kc
        wc = np.ascontiguousarray(w[:, sl]).reshape(KC, 128, D)
        return np.ascontiguousarray(wc.transpose(1, 0, 2).reshape(128, KC * D)).astype(bf16)

    in_maps = []
    for c in range(NCORES):
        sl = slice(c * D, (c + 1) * D)
        m = {
            "hT": hT,
            "wq": pack_w(Wq, sl),
            "wk": pack_w(Wk, sl),
            "wv": pack_w(Wv, sl),
        }
        if use_bias:
            m["bq"] = np.ascontiguousarray(bq[sl].reshape(D, 1))
            m["bk"] = np.ascontiguousarray(bk[sl].reshape(D, 1))
            m["bv"] = np.ascontiguousarray(bv[sl].reshape(D, 1))
        if use_mask:
            # [B, S] -> [128, B, NKT]: partition p holds key kt*128+p
            m["mask"] = np.ascontiguousarray(
                mask.reshape(B, NKT, 128).transpose(2, 0, 1)
            )
        in_maps.append(m)

    res = run_bass_kernel_spmd(
        nc, in_maps, core_ids=list(range(NCORES)), trace=_trace
    )
    # assemble: core c's [T, 128] token-major slice -> cols c*128:(c+1)*128
    full = np.concatenate([res.results[c]["out"] for c in range(NCORES)], axis=1)
    out = np.ascontiguousarray(full).reshape(B, S, H).astype(np.float32)
    if _trace:
        return out, res
    return out


# revision 24
# speedup vs baseline: 1.0032x; 1.0032x over previous
"""BertSelfAttention on 8 Trainium2 NeuronCores (Bass/Tile).

Sharding: tensor-parallel over heads. 16 heads / 8 cores = 2 heads (128
head-dim columns) per core. Each core computes the Q/K/V projections for
its 128 output dims over all 4096 tokens, then attention for its 2 heads
over both batches, producing a [4096, 128] token-major slice of the
output. The host transposes hidden_states once (and casts to bf16), feeds
every core the same [1024, 4096] activation matrix plus its private
weight slice, and reassembles the full [2, 2048, 1024] output by
concatenating the 8 column slices.

Device-side layout (driven by the cost model: a matmul charges
out_free_size cycles per contraction pass, so output partitions must be
maximized on every matmul):
  - Q,K are produced d-major [dim, token] in fp32r. QK^T yields scores
    TRANSPOSED, S^T[key, query], keys on partitions: 512-row matmuls at
    full fp32r rate.
  - exp() runs on ScalarE (the critical engine: 16.8M exps = ~133us);
    output is fp16 (exp(s/8) spans ~[0.1, 7.4], comfortably fp16).
  - V is projected d-major (fp16) then PE-transposed to token-major
    tiles with a ones-column appended per head: vtm[(b,kt)] =
    [128 keys, 65+65].
  - PV is oriented with QUERIES on output partitions: stationary =
    probs slice [128 keys, 128 queries], moving = vtm [128 keys, 65]
    -> ctx^T psum [128 q, 65] where column 64 is the softmax
    denominator. 65 charged rows/matmul instead of 512: PV costs 67k
    rows instead of 131k, and the denominator is per-PARTITION, so
    normalization is a plain DVE tensor_scalar multiply -- no
    reciprocal-broadcast matmuls at all.
  - hidden/weights are bf16 (halves the dominant 16MB input DMA);
    Q/K/scores stay fp32 so softmax inputs are near-exact.
  - Emission order IS the per-engine schedule. The kernel is
    software-pipelined so ScalarE (exp) never starves: each chunk's
    QK for chunk N+1 is emitted before chunk N's PV, projections ride
    as fillers in PE slack, and each block's first QK is hoisted into
    the previous block's last chunk.
"""

import numpy as np

import concourse.tile as tile
from concourse import bacc, mybir
from concourse.bass_utils import run_bass_kernel_spmd
from concourse.masks import make_identity

# Problem shape (hardcoded; harness contract)
B, S, H = 2, 2048, 1024
NUM_HEADS, DH = 16, 64
NCORES = 8
T = B * S                 # 4096 tokens total
D = H // NCORES           # 128 output dims per core (2 heads)
KC = H // 128             # 8 contraction chunks for projections
QB = 512                  # query-block width
NQB = S // QB             # 4 query blocks per batch
NTB = T // QB             # 8 projection token-blocks
NKT = S // 128            # 16 key tiles per batch
KTC = 2                   # key tiles per exp chunk (psum-budget limited)
NQT = QB // 128           # 4 query sub-tiles of 128 per block
SCALE = 1.0 / float(np.sqrt(DH))

F32 = mybir.dt.float32
F32R = mybir.dt.float32r
F16 = mybir.dt.float16
BF16 = mybir.dt.bfloat16
EXP = mybir.ActivationFunctionType.Exp


def build(use_mask: bool, use_bias: bool, reps: int = 1):
    nc = bacc.Bacc("TRN2", target_bir_lowering=False)

    hT = nc.dram_tensor("hT", [H, T], BF16, kind="ExternalInput")
    # weights pre-shuffled on host to [128, KC*D] so the DMA moves 2KB
    # contiguous lines per partition (full bus efficiency)
    wq = nc.dram_tensor("wq", [128, KC * D], BF16, kind="ExternalInput")
    wk = nc.dram_tensor("wk", [128, KC * D], BF16, kind="ExternalInput")
    wv = nc.dram_tensor("wv", [128, KC * D], BF16, kind="ExternalInput")
    if use_bias:
        bq = nc.dram_tensor("bq", [D, 1], F32, kind="ExternalInput")
        bk = nc.dram_tensor("bk", [D, 1], F32, kind="ExternalInput")
        bv = nc.dram_tensor("bv", [D, 1], F32, kind="ExternalInput")
    if use_mask:
        # host pre-transposes to [128, B, NKT] so the DMA is contiguous
        mask = nc.dram_tensor("mask", [128, B, NKT], F32, kind="ExternalInput")
    # token-major output: rows are tokens, cols are this core's 128 dims
    out = nc.dram_tensor("out", [T, D], F32, kind="ExternalOutput")

    with tile.TileContext(nc) as tc:
        with (
            tc.tile_pool(name="consts", bufs=1) as consts,
            tc.tile_pool(name="qkv", bufs=1) as qkvp,
            tc.tile_pool(name="ht", bufs=2) as htp,
            tc.tile_pool(name="vtm", bufs=2) as vtmp,
            tc.tile_pool(name="e", bufs=22) as ep,
            tc.tile_pool(name="small", bufs=4) as smallp,
            tc.tile_pool(name="pp", bufs=2, space="PSUM") as pp,
            tc.tile_pool(name="qk", bufs=2, space="PSUM") as qkp,
            tc.tile_pool(name="pv", bufs=2, space="PSUM") as pvp,
        ):
            # ---- constants ----
            w_sb = {}
            b_sb = {}
            w_dram = {"q": wq, "k": wk, "v": wv}
            for name in ("q", "k", "v"):
                w_sb[name] = consts.tile(
                    [128, KC, D], BF16, tag=f"w{name}", name=f"w{name}"
                )

            def load_w(name):
                nc.sync.dma_start(
                    out=w_sb[name][:],
                    in_=w_dram[name].rearrange("p (kc d) -> p kc d", kc=KC),
                )

            # DMA order: the sync queue drains in emission order. The first
            # K-projection half needs wk + the first half (kc 0..3) of hT
            # block 0, the Q half needs wq; wv is only needed later.
            hts = {}
            hT_r = hT.rearrange("(kc p) t -> p kc t", p=128)

            # issue the first three DMAs from three different engine queues
            # so their descriptor-generation pipelines overlap and the first
            # transfer starts ~0.6us earlier
            load_w("k")
            ht0 = htp.tile([128, KC, QB], BF16, tag="ht", name="ht")
            nc.sync.dma_start(
                out=ht0[:, 0 : KC // 2, :], in_=hT_r[:, 0 : KC // 2, 0:QB]
            )
            load_w("q")
            nc.sync.dma_start(
                out=ht0[:, KC // 2 :, :], in_=hT_r[:, KC // 2 :, 0:QB]
            )
            hts[0] = ht0
            load_w("v")

            if use_bias:
                for name, bt in (("q", bq), ("k", bk), ("v", bv)):
                    b_t = consts.tile([128, 1], F32, tag=f"b{name}")
                    nc.sync.dma_start(out=b_t[:], in_=bt[:])
                    b_sb[name] = b_t
            ident = consts.tile([128, 128], F16, tag="ident")
            make_identity(nc, ident[:])
            ones_st = consts.tile([128, 2], F16, tag="onesst")
            nc.vector.memset(ones_st[:], 1.0)
            # constant zero tile for p-state keeper matmuls (no DMA dep)
            dum = consts.tile([128, 128], BF16, tag="dum")
            nc.vector.memset(dum[:], 0.0)
            if use_mask:
                mask_sb = consts.tile([128, B, NKT], F32, tag="mask")
                nc.sync.dma_start(out=mask_sb[:], in_=mask[:])

            # per-token-block tiles so attention dependencies are precise
            Qts = [qkvp.tile([128, QB], F32R, tag=f"Qd{i}", name=f"Qd{i}") for i in range(NTB)]
            Kts = [qkvp.tile([128, QB], F32R, tag=f"Kd{i}", name=f"Kd{i}") for i in range(NTB)]
            Vts = [qkvp.tile([128, QB], F16, tag=f"Vd{i}", name=f"Vd{i}") for i in range(NTB)]

            def proj_load(tb):
                t0 = tb * QB
                ht_t = htp.tile([128, KC, QB], BF16, tag="ht", name="ht")
                hT_r = hT.rearrange("(kc p) t -> p kc t", p=128)
                nc.sync.dma_start(out=ht_t[:], in_=hT_r[:, :, t0 : t0 + QB])
                return ht_t

            def proj_group(tb, name):
                dest = {"q": Qts, "k": Kts, "v": Vts}[name][tb]
                ps = pp.tile([128, QB], F32, tag="pp", name="ps")
                for kc in range(KC):
                    nc.tensor.matmul(
                        ps[:],
                        w_sb[name][:, kc, :],
                        hts[tb][:, kc, :],
                        start=(kc == 0),
                        stop=(kc == KC - 1),
                    )
                if use_bias:
                    nc.vector.tensor_scalar_add(dest[:], ps[:], b_sb[name][:])
                else:
                    nc.vector.tensor_copy(dest[:], ps[:])

            # V token-major tiles per k-tile, 130 wide:
            # [h0 dims 0-63 | ones | h1 dims 0-63 | ones]; the ones column
            # makes the PV matmul also produce the softmax denominator in
            # psum column 64 (per query partition).
            vtms = {}

            def transpose_v(b, kt):
                g0 = b * S + kt * 128
                tbi, off = divmod(g0, QB)
                vt = vtmp.tile([128, 130], F16, tag=f"vtm{kt}", name=f"vtm{kt}")
                nc.vector.tensor_copy(
                    vt[:, 64::65].rearrange("p (a o) -> p a o", o=1),
                    ones_st[:, 0:2].rearrange("p (a o) -> p a o", o=1),
                )
                tps = pp.tile([128, 128], F16, tag="pp", name="tps")
                nc.tensor.transpose(tps[:], Vts[tbi][:, off : off + 128], ident[:])
                nc.vector.tensor_copy(
                    vt[:].rearrange("p (g c) -> p g c", g=2)[:, :, 0:64],
                    tps.rearrange("p (g c) -> p g c", g=2),
                )
                vtms[(b, kt)] = vt

            # ---- explicit software-pipelined emission ----

            def attn_open(b, qb):
                # per head: one bank [128, NQT, 65] = ctx^T rows (queries on
                # partitions) + denominator column 64, 4 query-subtile
                # accumulation groups. start_tensor_calc resets the WHOLE
                # psum bank on hardware, so the sub-bank qt groups cannot
                # each use start=True: zero the bank once (DVE) and let
                # every PV matmul accumulate.
                ctx_ps = []
                for h in range(2):
                    t = pvp.tile([128, NQT, 65], F32, tag="ctx", name=f"ctx{h}")
                    nc.vector.memset(t[:], 0.0)
                    ctx_ps.append(t)
                return (b, qb, ctx_ps)

            def attn_qk(state, ktc):
                """Emit one chunk's QK^T matmuls. Returns score psum tiles."""
                b, qb, _ = state
                q0 = b * S + qb * QB
                sps = [
                    qkp.tile([128, KTC, QB], F32, tag="sps", name=f"sps{h}")
                    for h in range(2)
                ]
                for j in range(KTC):
                    kt = ktc * KTC + j
                    tbi, off = divmod(b * S + kt * 128, QB)
                    for h in (0, 1):
                        nc.tensor.matmul(
                            sps[h][:, j, :],
                            Kts[tbi][h * 64 : (h + 1) * 64, off : off + 128],
                            Qts[q0 // QB][h * 64 : (h + 1) * 64, :],
                            start=True,
                            stop=True,
                        )
                return sps

            def attn_exp(state, ktc, sps):
                """exp on ScalarE -> fp16 prob tiles (the critical engine)."""
                b, qb, _ = state
                ets = []
                for h in (0, 1):
                    et = ep.tile([128, KTC, QB], F16, tag="e", name=f"et{h}")
                    if use_mask:
                        for j in range(KTC):
                            kt = ktc * KTC + j
                            nc.scalar.activation(
                                et[:, j, :],
                                sps[h][:, j, :],
                                EXP,
                                bias=mask_sb[:, b, kt : kt + 1],
                                scale=SCALE,
                            )
                    else:
                        nc.scalar.activation(et[:], sps[h][:], EXP, scale=SCALE)
                    ets.append(et)
                return ets

            def attn_pv(state, ktc, ets):
                b, qb, ctx_ps = state
                for j in range(KTC):
                    kt = ktc * KTC + j
                    for h in (0, 1):
                        for qt in range(NQT):
                            nc.tensor.matmul(
                                ctx_ps[h][:, qt, :],
                                ets[h][:, j, qt * 128 : (qt + 1) * 128],
                                vtms[(b, kt)][:, h * 65 : (h + 1) * 65],
                                start=False,
                                stop=(kt == NKT - 1),
                                skip_group_check=True,
                            )

            # exp-stream / pv-stream decoupling: exps for block E run while
            # PVs for the PREVIOUS block P consume parked prob tiles. The
            # two streams only meet through the et pool, so ScalarE never
            # waits for PV progress and the PE always has parked PV work.
            park = {}  # (b, qb) -> {ktc: ets}

            def exp_chunk(b, qb, ktc):
                stv = (b, qb, None)
                sps = attn_qk(stv, ktc)
                ets = attn_exp(stv, ktc, sps)
                park.setdefault((b, qb), {})[ktc] = ets

            def pv_chunk(state, ktc):
                b, qb, _ = state
                attn_pv(state, ktc, park[(b, qb)].pop(ktc))

            def attn_close(state, last=False):
                """Emit the reciprocals now (DVE, off the PE/ACT paths) and
                return a thunk with the normalize + output-DMA tail. For the
                final block (no more exps) half the normalize runs on the
                now-idle ScalarE to shorten the serial tail."""
                b, qb, ctx_ps = state
                recs = []
                for h in (0, 1):
                    rec = smallp.tile([128, NQT, 1], F32, tag="rec", name=f"rec{h}")
                    nc.vector.reciprocal(rec[:], ctx_ps[h][:, :, 64:65])
                    recs.append(rec)

                def finish():
                    ot = smallp.tile([128, NQT, 2, 64], F32, tag="ot", name="ot", bufs=2)
                    for qt in range(NQT):
                        for h in (0, 1):
                            if last and h == 1:
                                nc.scalar.mul(
                                    ot[:, qt, h, :],
                                    ctx_ps[h][:, qt, 0:64],
                                    recs[h][:, qt, :],
                                )
                            else:
                                nc.vector.tensor_scalar_mul(
                                    ot[:, qt, h, :],
                                    ctx_ps[h][:, qt, 0:64],
                                    recs[h][:, qt, :],
                                )
                    tb0 = (b * S + qb * QB) // 128
                    out_r = out.rearrange("(tb p) d -> p tb d", p=128)
                    nc.sync.dma_start(
                        out=out_r[:, tb0 : tb0 + NQT, :],
                        in_=ot[:].rearrange("p a b c -> p a (b c)"),
                    )

                return finish

            # --- the pipeline driver ---
            # PE p-state: the cost model drops the PE clock whenever the
            # engine idles (the ramp needs ~3us of continuous execution to
            # reach full rate). Tiny dummy matmuls on a memset tile (no DMA
            # dependency) keep the PE continuously busy from t~0.3us across
            # the initial DMA waits so the first REAL matmuls run at full
            # rate immediately.
            warm = qkp.tile([128, KTC, QB], F32, tag="sps", name="warm")

            def dummies(n):
                def f():
                    for _ in range(n):
                        nc.tensor.matmul(
                            warm[0:64, 0, 0:64],
                            dum[:, 0:64],
                            dum[:, 0:64],
                            start=True,
                            stop=True,
                        )

                return f

            dummies(110)()

            def load(tb):
                def f():
                    hts[tb] = proj_load(tb)

                return f

            def grp(tb, n):
                return lambda: proj_group(tb, n)

            def grp_halves(tb, n):
                """Split one projection group into two 4-kc emission halves
                (same psum accumulation bracket) so a filler never injects
                more than ~0.9us of PE work between attention chunks."""
                stash = {}

                def h1():
                    dest = {"q": Qts, "k": Kts, "v": Vts}[n][tb]
                    ps = pp.tile([128, QB], F32, tag="pp", name="ps")
                    stash["ps"], stash["dest"] = ps, dest
                    for kc in range(KC // 2):
                        nc.tensor.matmul(
                            ps[:],
                            w_sb[n][:, kc, :],
                            hts[tb][:, kc, :],
                            start=(kc == 0),
                            stop=False,
                        )

                def h2():
                    ps, dest = stash["ps"], stash["dest"]
                    for kc in range(KC // 2, KC):
                        nc.tensor.matmul(
                            ps[:],
                            w_sb[n][:, kc, :],
                            hts[tb][:, kc, :],
                            start=False,
                            stop=(kc == KC - 1),
                        )
                    if use_bias:
                        nc.vector.tensor_scalar_add(dest[:], ps[:], b_sb[n][:])
                    else:
                        nc.vector.tensor_copy(dest[:], ps[:])

                return h1, h2

            def seq(*fs):
                def f():
                    for g in fs:
                        g()

                return f

            def tr2(b_, k_):
                def f():
                    transpose_v(b_, k_)
                    transpose_v(b_, k_ + 1)

                return f

            def dmy():
                # p-state keeper for chunks with no real filler work: ~640ns
                # of junk matmuls into a rotating pp slot so the PE never
                # idles (an idle PE drops to the slow clock for ~3us).
                dps = pp.tile([64, QB], F32, tag="pp", name="dps")
                for _ in range(3):
                    nc.tensor.matmul(
                        dps[:],
                        dum[:, 0:64],
                        w_sb["q"][:, 0:4, :].rearrange("p a b -> p (a b)"),
                        start=True,
                        stop=True,
                    )

            none = lambda: None

            def emit_pass():
                # --- priming: exps for (0,0) AND (0,1) interleave as K/V
                # blocks get projected, so ScalarE is nearly saturated even
                # while the PE is projection-bound. (0,1)'s probs are parked
                # in the et pool; its PVs run in the first pair below.
                # Dummy batches bridge DMA waits so the PE p-state never
                # drops before/between the first real matmuls.
                st00 = attn_open(0, 0)
                k1, k2 = grp_halves(0, "k")
                q1, q2 = grp_halves(0, "q")
                v1, v2 = grp_halves(0, "v")
                k1(); q1()
                dummies(10)()
                k2(); q2()
                dummies(6)()
                exp_chunk(0, 0, 0)
                seq(v1, v2, tr2(0, 0))()
                exp_chunk(0, 0, 1)
                tr2(0, 2)()
                pv_chunk(st00, 0)
                for tb in range(1, NTB // B):
                    load(tb)()
                    if tb > 1:
                        # (0,1)'s chunk uses the PREVIOUS stage's K, so its
                        # QK/exp can feed ScalarE before this stage's
                        # projections finish
                        exp_chunk(0, 1, 2 * (tb - 1))
                    grp(tb, "k")()
                    grp(tb, "v")()
                    for kt in range(4 * tb, 4 * tb + 4):
                        transpose_v(0, kt)
                    if tb in (1, 2):
                        grp(tb, "q")()
                    if tb == NTB // B - 1:
                        load(NTB // B)()
                    exp_chunk(0, 0, 2 * tb)
                    if tb == 1:
                        exp_chunk(0, 1, 0)
                    pv_chunk(st00, 2 * tb - 1)
                    exp_chunk(0, 0, 2 * tb + 1)
                    exp_chunk(0, 1, 2 * (tb - 1) + 1)
                    pv_chunk(st00, 2 * tb)
                q31, q32 = grp_halves(NTB // B - 1, "q")
                exp_chunk(0, 1, 6)
                q31()
                exp_chunk(0, 1, 7)
                q32()
                pv_chunk(st00, 7)
                fin = attn_close(st00)

                # --- steady state: pairs (P, E) — P's parked PVs + E's
                # QK/exps per slot; batch-1 projections/transposes/loads
                # ride as per-slot fillers, q-projections just-in-time,
                # dummy batches keep the PE p-state up where there's no
                # real filler work.
                q41, q42 = grp_halves(4, "q"); k41, k42 = grp_halves(4, "k"); v41, v42 = grp_halves(4, "v")
                q51, q52 = grp_halves(5, "q"); k51, k52 = grp_halves(5, "k"); v51, v52 = grp_halves(5, "v")
                q61, q62 = grp_halves(6, "q"); k61, k62 = grp_halves(6, "k"); v61, v62 = grp_halves(6, "v")
                q71, q72 = grp_halves(7, "q"); k71, k72 = grp_halves(7, "k"); v71, v72 = grp_halves(7, "v")
                pairs = [
                    ((0, 1), (0, 2), [q41, q42, k41, k42, v41,
                                      seq(v42, load(5)), tr2(1, 0), tr2(1, 2)]),
                    ((0, 2), (0, 3), [q51, q52, k51, k52, v51,
                                      seq(v52, load(6)), tr2(1, 4),
                                      seq(tr2(1, 6), load(7))]),
                    ((0, 3), (1, 0), [k61, k62, v61, v62, k71, k72,
                                      tr2(1, 8), tr2(1, 10)]),
                    ((1, 0), (1, 1), [q61, q62, v71, seq(v72, tr2(1, 12)),
                                      tr2(1, 14), q71, q72, dmy]),
                    ((1, 1), (1, 2), [dmy] * 8),
                ]
                for (pb, pqb), (eb, eqb), fillers in pairs:
                    stP = attn_open(pb, pqb)
                    for c in range(NKT // KTC):
                        fillers[c]()
                        exp_chunk(eb, eqb, c)
                        if c == 0:
                            fin()
                        pv_chunk(stP, c)
                    fin = attn_close(stP)

                # --- final pair: P=(1,2) drains its parked PVs early (2 per
                # slot), closes mid-pair, then E=(1,3) opens (ctx banks
                # freed) and drains its own parked PVs in the last slots so
                # the tail after the final exp is just 2 PV batches + the
                # (ACT/DVE-split) normalize.
                stP = attn_open(1, 2)
                stE = None
                finP = None
                for c in range(NKT // KTC):
                    exp_chunk(1, 3, c)
                    if c == 0:
                        fin()
                    if c < 4:
                        pv_chunk(stP, 2 * c)
                        pv_chunk(stP, 2 * c + 1)
                    elif c == 4:
                        finP = attn_close(stP)
                        dmy()
                    elif c == 5:
                        finP()
                        dmy()
                    elif c == 6:
                        stE = attn_open(1, 3)
                        pv_chunk(stE, 0)
                        pv_chunk(stE, 1)
                        pv_chunk(stE, 2)
                    else:
                        pv_chunk(stE, 3)
                        pv_chunk(stE, 4)
                        pv_chunk(stE, 5)
                pv_chunk(stE, 6)
                pv_chunk(stE, 7)
                finE = attn_close(stE, last=True)
                finE()

            for _ in range(reps):
                emit_pass()
    nc.compile()
    return nc


_BUILD_CACHE = {}


def _get_nc(use_mask, use_bias):
    key = (use_mask, use_bias)
    if key not in _BUILD_CACHE:
        _BUILD_CACHE[key] = build(use_mask, use_bias)
    return _BUILD_CACHE[key]


def kernel(hidden_states, attention_mask, Wq, bq, Wk, bk, Wv, bv, _trace=False):
    import ml_dtypes

    bf16 = ml_dtypes.bfloat16

    hidden = np.ascontiguousarray(np.asarray(hidden_states, dtype=np.float32))
    mask = np.asarray(attention_mask, dtype=np.float32).reshape(B, S)
    Wq = np.asarray(Wq, dtype=np.float32)
    Wk = np.asarray(Wk, dtype=np.float32)
    Wv = np.asarray(Wv, dtype=np.float32)
    bq = np.asarray(bq, dtype=np.float32)
    bk = np.asarray(bk, dtype=np.float32)
    bv = np.asarray(bv, dtype=np.float32)

    use_mask = bool(np.any(mask != 0.0))
    use_bias = bool(np.any(bq != 0.0) or np.any(bk != 0.0) or np.any(bv != 0.0))
    nc = _get_nc(use_mask, use_bias)

    hT = np.ascontiguousarray(hidden.reshape(T, H).T).astype(bf16)  # [H, T]

    def pack_w(w, sl):
        # [H, D] slice -> [128, KC*D]: row p holds w[kc*128+p, :] for all kc
        wc = np.ascontiguousarray(w[:, sl]).reshape(KC, 128, D)
        return np.ascontiguousarray(wc.transpose(1, 0, 2).reshape(128, KC * D)).astype(bf16)

    in_maps = []
    for c in range(NCORES):
        sl = slice(c * D, (c + 1) * D)
        m = {
            "hT": hT,
            "wq": pack_w(Wq, sl),
            "wk": pack_w(Wk, sl),
            "wv": pack_w(Wv, sl),
        }
        if use_bias:
            m["bq"] = np.ascontiguousarray(bq[sl].reshape(D, 1))
            m["bk"] = np.ascontiguousarray(bk[sl].reshape(D, 1))
            m["bv"] = np.ascontiguousarray(bv[sl].reshape(D, 1))
        if use_mask:
            # [B, S] -> [128, B, NKT]: partition p holds key kt*128+p
            m["mask"] = np.ascontiguousarray(
                mask.reshape(B, NKT, 128).transpose(2, 0, 1)
            )
        in_maps.append(m)

    res = run_bass_kernel_spmd(
        nc, in_maps, core_ids=list(range(NCORES)), trace=_trace
    )
    # assemble: core c's [T, 128] token-major slice -> cols c*128:(c+1)*128
    full = np.concatenate([res.results[c]["out"] for c in range(NCORES)], axis=1)
    out = np.ascontiguousarray(full).reshape(B, S, H).astype(np.float32)
    if _trace:
        return out, res
    return out
